# revision 11
# baseline (speedup 1.0000x reference)
"""Trainium2 Bass kernel for MipRayMarcher2 (NeRF ray marching).

Full shapes: colors/sample_coordinates [4,16384,96,3], densities/depths
[4,16384,96,1]. Rays are flattened (B*R = 65536) and sharded contiguously
across 8 NeuronCores (8192 rays/core).

Per-core layout: 128 rays on partitions, K sample-windows of 96 along the
free axis per "supertile" (128*K rays). The per-supertile dataflow is
software-pipelined: each stage is emitted one loop iteration after its
producers, so no engine ever waits on a same-iteration cross-engine dep.

Math restructuring (validated vs reference to ~4e-6 rel err):
  a      = sigma_s + sigma_{s+1}
  sp     = softplus(0.5*a) = Ln(Exp(0.5*a) + 1)      (one ACT table set)
  x      = sp * (d_{s+1} - d_s)
  e      = Exp(-x)                                   (= 1 - alpha)
  T_s    = cumprod(e) via one tensor_tensor_scan per supertile with a
           per-window reset mask (boundary slot: e=0, mask=1 -> state=1)
  w_s    = T_s - T_{s+1}            (weights output, packed for DMA)
  wtot   = 1 - T_95                 (telescoped sum of weights)
  tau    = T_94
  g_s    = w_s + w_{s-1}  (window edges copied), composite_X = sum_s 0.5*g_s*X_s
  (0.5 folded into scalar_tensor_tensor's scalar; per-ray sums via accum_out)

Engine split: Pool (gpsimd) runs the plain tensor_tensor chain, ACT runs the
transcendentals + small copies, DVE runs the scan + all accumulating
scalar_tensor_tensor ops, PE idle, sync/HWDGE runs DMA.
"""

from contextlib import ExitStack

import numpy as np

import concourse.bass as bass
from concourse import bacc
import concourse.tile as tile
from concourse import mybir
from concourse.bass_utils import run_bass_kernel_spmd

# Problem constants (hardcoded; kernel.py must be self-contained)
B, R, S = 4, 16384, 96
N_CORES = 8
RAYS_TOTAL = B * R                 # 65536
RAYS_PER_CORE = RAYS_TOTAL // N_CORES  # 8192

F32 = mybir.dt.float32
ALU = mybir.AluOpType
ACTF = mybir.ActivationFunctionType

# Keep Exp and Ln in ONE activation table set (natural_log_exp_and_others) so
# the table-load pass doesn't alternate sets (each switch costs ~2.7us).
_orig_get_tables = bacc.get_activation_tables


def _patched_get_tables(arch):
    tables = _orig_get_tables(arch)
    for name, funcs in tables.items():
        if name != "natural_log_exp_and_others":
            funcs.discard(ACTF.Exp)
            funcs.discard(ACTF.Ln)
    return tables


bacc.get_activation_tables = _patched_get_tables


def build_marcher(n_rays: int, K: int = 4) -> bass.Bass:
    """Build the Bass module for one core processing n_rays rays.

    Ray order within a core: ray = t*(128*K) + p*K + k
    (t = supertile, p = partition, k = window) so each partition reads a
    contiguous run of K rays per DMA.
    """
    P = 128
    rays_per_tile = P * K
    assert n_rays % rays_per_tile == 0
    n_super = n_rays // rays_per_tile
    W = K * S

    nc = bacc.Bacc("TRN2")

    colors_d = nc.dram_tensor("colors", [n_rays, 3 * S], F32, kind="ExternalInput")
    coords_d = nc.dram_tensor("coords", [n_rays, 3 * S], F32, kind="ExternalInput")
    dens_d = nc.dram_tensor("dens", [n_rays, S], F32, kind="ExternalInput")
    depths_d = nc.dram_tensor("depths", [n_rays, S], F32, kind="ExternalInput")

    weights_d = nc.dram_tensor("weights", [n_rays, S - 1], F32, kind="ExternalOutput")
    # smalls: rgb0 rgb1 rgb2  pnt0 pnt1 pnt2  depth  wtot  tau
    smalls_d = nc.dram_tensor("smalls", [n_rays, 9], F32, kind="ExternalOutput")

    colors_v = colors_d[:].rearrange("(t p k) j -> t p (k j)", p=P, k=K)
    coords_v = coords_d[:].rearrange("(t p k) j -> t p (k j)", p=P, k=K)
    dens_v = dens_d[:].rearrange("(t p k) j -> t p (k j)", p=P, k=K)
    depths_v = depths_d[:].rearrange("(t p k) j -> t p (k j)", p=P, k=K)
    weights_v = weights_d[:].rearrange("(t p k) s -> t p k s", p=P, k=K)
    smalls_v = smalls_d[:].rearrange("(t p k) s -> t p k s", p=P, k=K)

    # pipeline state per in-flight supertile
    st = {}

    with tile.TileContext(nc) as tc, ExitStack() as ctx:
        singles = ctx.enter_context(tc.tile_pool(name="singles", bufs=1))
        p_dens = ctx.enter_context(tc.tile_pool(name="p_dens", bufs=3))
        p_depths = ctx.enter_context(tc.tile_pool(name="p_depths", bufs=9))
        p_cc = ctx.enter_context(tc.tile_pool(name="p_cc", bufs=6))
        p_a = ctx.enter_context(tc.tile_pool(name="p_a", bufs=3))
        p_u = ctx.enter_context(tc.tile_pool(name="p_u", bufs=2))
        p_sp = ctx.enter_context(tc.tile_pool(name="p_sp", bufs=3))
        p_delta = ctx.enter_context(tc.tile_pool(name="p_delta", bufs=4))
        p_x = ctx.enter_context(tc.tile_pool(name="p_x", bufs=3))
        p_eb = ctx.enter_context(tc.tile_pool(name="p_eb", bufs=4))
        p_trans = ctx.enter_context(tc.tile_pool(name="p_trans", bufs=4))
        p_w = ctx.enter_context(tc.tile_pool(name="p_w", bufs=4))
        p_g = ctx.enter_context(tc.tile_pool(name="p_g", bufs=3))
        p_smalls = ctx.enter_context(tc.tile_pool(name="p_smalls", bufs=3))
        dummies = ctx.enter_context(tc.tile_pool(name="dummies", bufs=4))

        # scan reset mask: 1.0 at each window start, 0.0 elsewhere (constant)
        mask = singles.tile([P, W], F32)
        nc.gpsimd.memset(mask[:], 0.0)
        nc.gpsimd.memset(mask[:].rearrange("p (k s) -> p k s", s=S)[:, :, 0:1], 1.0)

        DEPTH = 9
        for it in range(n_super + DEPTH - 1):
            # ---- stage 0: DMA dens/depths for supertile t0
            t0 = it
            if t0 < n_super:
                d = st.setdefault(t0, {})
                d["dens"] = p_dens.tile([P, W], F32, name="dens")
                d["depths"] = p_depths.tile([P, W], F32, name="depths")
                nc.sync.dma_start(out=d["dens"][:], in_=dens_v[t0])
                nc.sync.dma_start(out=d["depths"][:], in_=depths_v[t0])

            # ---- stage 1: pool a, delta
            t1 = it - 1
            if 0 <= t1 < n_super:
                d = st[t1]
                dens3 = d["dens"][:].rearrange("p (k s) -> p k s", s=S)
                depths3 = d["depths"][:].rearrange("p (k s) -> p k s", s=S)
                d["a"] = p_a.tile([P, W], F32, name="a")
                a3 = d["a"][:].rearrange("p (k s) -> p k s", s=S)
                nc.gpsimd.tensor_add(
                    a3[:, :, 0:95], dens3[:, :, 0:95], dens3[:, :, 1:96]
                )
                d["delta"] = p_delta.tile([P, W], F32, name="delta")
                delta3 = d["delta"][:].rearrange("p (k s) -> p k s", s=S)
                nc.gpsimd.tensor_sub(
                    delta3[:, :, 0:95], depths3[:, :, 1:96], depths3[:, :, 0:95]
                )

            # ---- stage 2: ACT u = Exp(0.5 a), sp = Ln(u + 1)
            t2 = it - 2
            if 0 <= t2 < n_super:
                d = st[t2]
                a3 = d["a"][:].rearrange("p (k s) -> p k s", s=S)
                d["u"] = p_u.tile([P, W], F32, name="u")
                u3 = d["u"][:].rearrange("p (k s) -> p k s", s=S)
                nc.scalar.activation(u3[:, :, 0:95], a3[:, :, 0:95], ACTF.Exp, scale=0.5)
                d["sp"] = p_sp.tile([P, W], F32, name="sp")
                sp3 = d["sp"][:].rearrange("p (k s) -> p k s", s=S)
                nc.scalar.activation(sp3[:, :, 0:95], u3[:, :, 0:95], ACTF.Ln, bias=1.0)

            # ---- stage 3: pool x = sp*delta, memset eb boundary; DMA colors/coords
            t3 = it - 3
            if 0 <= t3 < n_super:
                d = st[t3]
                sp3 = d["sp"][:].rearrange("p (k s) -> p k s", s=S)
                delta3 = d["delta"][:].rearrange("p (k s) -> p k s", s=S)
                d["x"] = p_x.tile([P, W], F32, name="x")
                x3 = d["x"][:].rearrange("p (k s) -> p k s", s=S)
                nc.gpsimd.tensor_mul(
                    x3[:, :, 0:95], sp3[:, :, 0:95], delta3[:, :, 0:95]
                )
                d["eb"] = p_eb.tile([P, W], F32, name="eb")
                eb3 = d["eb"][:].rearrange("p (k s) -> p k s", s=S)
                nc.gpsimd.memset(eb3[:, :, 0:1], 0.0)

                d["colors"] = p_cc.tile([P, 3 * W], F32, name="colors", tag="colors")
                d["coords"] = p_cc.tile([P, 3 * W], F32, name="coords", tag="coords")
                nc.sync.dma_start(out=d["colors"][:], in_=colors_v[t3])
                nc.sync.dma_start(out=d["coords"][:], in_=coords_v[t3])

            # ---- stage 4: ACT eb = Exp(-x) into slots 1..95
            t4 = it - 4
            if 0 <= t4 < n_super:
                d = st[t4]
                x3 = d["x"][:].rearrange("p (k s) -> p k s", s=S)
                eb3 = d["eb"][:].rearrange("p (k s) -> p k s", s=S)
                nc.scalar.activation(
                    eb3[:, :, 1:96], x3[:, :, 0:95], ACTF.Exp, scale=-1.0
                )

            # ---- stage 5: DVE masked scan -> trans
            t5 = it - 5
            if 0 <= t5 < n_super:
                d = st[t5]
                d["trans"] = p_trans.tile([P, W], F32, name="trans")
                nc.vector.tensor_tensor_scan(
                    d["trans"][:], d["eb"][:], mask[:], 0.0,
                    op0=ALU.mult, op1=ALU.add,
                )

            # ---- stage 6: pool w = T_s - T_{s+1} (packed), g = w + w_shift
            t6 = it - 6
            if 0 <= t6 < n_super:
                d = st[t6]
                trans3 = d["trans"][:].rearrange("p (k s) -> p k s", s=S)
                d["w"] = p_w.tile([P, K * (S - 1)], F32, name="w")
                w3 = d["w"][:].rearrange("p (k s) -> p k s", s=S - 1)
                nc.gpsimd.tensor_sub(
                    w3[:, :, :], trans3[:, :, 0:95], trans3[:, :, 1:96]
                )
                d["g"] = p_g.tile([P, W], F32, name="g")
                g3 = d["g"][:].rearrange("p (k s) -> p k s", s=S)
                nc.gpsimd.tensor_add(g3[:, :, 1:95], w3[:, :, 1:95], w3[:, :, 0:94])
                nc.gpsimd.tensor_copy(out=g3[:, :, 0:1], in_=w3[:, :, 0:1])
                nc.gpsimd.tensor_copy(out=g3[:, :, 95:96], in_=w3[:, :, 94:95])

            # ---- stage 7: DVE weighted sums; ACT tau/wtot
            t7 = it - 7
            if 0 <= t7 < n_super:
                d = st[t7]
                g3 = d["g"][:].rearrange("p (k s) -> p k s", s=S)
                trans3 = d["trans"][:].rearrange("p (k s) -> p k s", s=S)
                depths3 = d["depths"][:].rearrange("p (k s) -> p k s", s=S)
                colors4 = d["colors"][:].rearrange("p (k s c) -> p k s c", k=K, s=S)
                coords4 = d["coords"][:].rearrange("p (k s c) -> p k s c", k=K, s=S)
                smalls = p_smalls.tile([P, K, 9], F32, name="smalls")
                d["smalls"] = smalls
                for k in range(K):
                    gk = g3[:, k, :]
                    for c in range(3):
                        nc.vector.scalar_tensor_tensor(
                            out=dummies.tile([P, S], F32, tag="dump", name="dump"),
                            in0=colors4[:, k, :, c],
                            scalar=0.5,
                            in1=gk,
                            op0=ALU.mult,
                            op1=ALU.mult,
                            accum_out=smalls[:, k, c : c + 1],
                        )
                    for c in range(3):
                        nc.vector.scalar_tensor_tensor(
                            out=dummies.tile([P, S], F32, tag="dump", name="dump"),
                            in0=coords4[:, k, :, c],
                            scalar=0.5,
                            in1=gk,
                            op0=ALU.mult,
                            op1=ALU.mult,
                            accum_out=smalls[:, k, 3 + c : 4 + c],
                        )
                    nc.vector.scalar_tensor_tensor(
                        out=dummies.tile([P, S], F32, tag="dump", name="dump"),
                        in0=depths3[:, k, :],
                        scalar=0.5,
                        in1=gk,
                        op0=ALU.mult,
                        op1=ALU.mult,
                        accum_out=smalls[:, k, 6:7],
                    )
                # wtot = 1 - T_95 (telescoped); tau = T_94
                nc.scalar.activation(
                    out=smalls[:, :, 7:8], in_=trans3[:, :, 95:96],
                    func=ACTF.Copy, bias=1.0, scale=-1.0,
                )
                nc.scalar.copy(out=smalls[:, :, 8:9], in_=trans3[:, :, 94:95])

            # ---- stage 8: DMA out
            t8 = it - 8
            if 0 <= t8 < n_super:
                d = st[t8]
                w3 = d["w"][:].rearrange("p (k s) -> p k s", s=S - 1)
                nc.sync.dma_start(out=weights_v[t8], in_=w3[:, :, :])
                nc.sync.dma_start(out=smalls_v[t8], in_=d["smalls"][:])
                del st[t8]

    nc.finalize()
    return nc


_BUILT = {}


def _get_built(n_rays: int, K: int):
    key = (n_rays, K)
    if key not in _BUILT:
        _BUILT[key] = build_marcher(n_rays, K)
    return _BUILT[key]


def run_sharded(colors, densities, depths, sample_coordinates, n_cores=N_CORES,
                K=4, trace=False, trace_kwargs=None):
    """Shard flat inputs across cores, run, return per-core results + perf."""
    rays_per_core = RAYS_TOTAL // n_cores

    cf = np.ascontiguousarray(colors, np.float32).reshape(RAYS_TOTAL, 3 * S)
    sf = np.ascontiguousarray(sample_coordinates, np.float32).reshape(RAYS_TOTAL, 3 * S)
    df = np.ascontiguousarray(densities, np.float32).reshape(RAYS_TOTAL, S)
    zf = np.ascontiguousarray(depths, np.float32).reshape(RAYS_TOTAL, S)

    nc = _get_built(rays_per_core, K)
    in_maps = []
    for i in range(n_cores):
        sl = slice(i * rays_per_core, (i + 1) * rays_per_core)
        in_maps.append(
            {"colors": cf[sl], "coords": sf[sl], "dens": df[sl], "depths": zf[sl]}
        )
    res = run_bass_kernel_spmd(
        nc,
        in_maps,
        core_ids=list(range(n_cores)),
        trace=trace,
        trace_kwargs=trace_kwargs or {},
    )
    return res


def kernel(colors, densities, depths, sample_coordinates, white_back):
    """Full-input entry point: returns the same tuple as reference()."""
    res = run_sharded(colors, densities, depths, sample_coordinates)

    weights = np.concatenate([r["weights"] for r in res.results], axis=0)
    smalls = np.concatenate([r["smalls"] for r in res.results], axis=0)

    composite_rgb = smalls[:, 0:3].reshape(B, R, 3)
    composite_point = smalls[:, 3:6].reshape(B, R, 3)
    composite_depth = smalls[:, 6:7].reshape(B, R, 1)
    weight_total = smalls[:, 7:8].reshape(B, R, 1)
    tau = smalls[:, 8:9].reshape(B, R, 1)
    weights = weights.reshape(B, R, S - 1, 1)

    depths_np = np.asarray(depths)
    composite_depth = np.nan_to_num(composite_depth, nan=np.inf)
    composite_depth = np.clip(composite_depth, depths_np.min(), depths_np.max())

    wb = np.asarray(white_back)
    if wb.item() != 0:
        composite_rgb = composite_rgb + 1.0 - weight_total

    return composite_rgb, composite_depth, weights, composite_point, tau
